# revision 2
# baseline (speedup 1.0000x reference)
"""EnergyAttention Trainium2 kernel v2 (8 NeuronCores, head-sharded).

Per core (2 heads): master Q kept as [q, hd] fp32 tiles; scores S^T[k, q]
via bf16 64-contraction matmuls (two heads via base-partition tile
placement); exp split between ACT (exact, fp8 out) and DVE (Schraudolph
int8 bitcast -> fp8e4); gradient via fp8 DoubleRow matmuls in flipped
orientation out=[q, hd+1] (full 128-partition PSUM writes, ones-column
value 10 folds the 0.1 step size); q-update fused with per-partition
reciprocal scalar; qT (scores rhs) refreshed by PE transposes.
Output written transposed [d, q] bf16; host sums partials.
"""

import numpy as np
import ml_dtypes

BF16 = ml_dtypes.bfloat16
F8E4 = ml_dtypes.float8_e4m3

N_CORES = 8
D = 1024
K = 4096
Q = 2048
H = 16
HD = 64
STEPS = 5
BETA = 1.0 / np.sqrt(np.float32(HD))  # 1/8

QB = 256
LOG2E = 1.4426950408889634

_CACHE = {}


def build_program(d=D, k=K, q=Q, steps=STEPS, n_cores=N_CORES, n_dve=7,
                  pop_rate=2, pops_first=False, grad_delay=2, dbg=()):
    """Build + compile the per-core Bass program. Returns the Bacc object."""
    from contextlib import ExitStack

    import concourse.tile as tile
    from concourse import bacc, mybir

    f32 = mybir.dt.float32
    bf16 = mybir.dt.bfloat16
    i8 = mybir.dt.int8
    f8e4 = mybir.dt.float8e4

    ndc = d // 128       # D chunks (contraction for projections)
    nkb = k // 512       # k blocks for K^T projection
    nkc = k // 128       # k chunks for the step loop
    npair = nkc // 2     # 256-k pairs for DoubleRow grad
    nqb = q // QB        # q blocks (512)
    nqt = q // 128       # q tiles (128)
    beta = float(1.0 / np.sqrt(np.float64(HD)))
    A8 = 8.0 * LOG2E * beta
    B8 = 8.0 * (7.0 - 0.043677) + 0.5  # centered Schraudolph + trunc bias

    EXP = mybir.ActivationFunctionType.Exp
    DR = mybir.MatmulPerfMode.DoubleRow
    MULT = mybir.AluOpType.mult
    ADD = mybir.AluOpType.add

    # k-pair indices whose exp runs on DVE (Schraudolph)
    dve_set = set()
    if n_dve > 0:
        idx = np.linspace(0, npair - 1, n_dve).round().astype(int)
        dve_set = {int(i) for i in idx}

    nc = bacc.Bacc("TRN2", target_bir_lowering=False, debug=False,
                   num_devices=n_cores)
    ctxT = nc.dram_tensor("ctxT", [d, k], f8e4, kind="ExternalInput").ap()
    tgtT = nc.dram_tensor("tgtT", [d, q], bf16, kind="ExternalInput").ap()
    wk = nc.dram_tensor("wk", [d, 128], bf16, kind="ExternalInput").ap()
    wq = nc.dram_tensor("wq", [d, 128], bf16, kind="ExternalInput").ap()
    woT = nc.dram_tensor("woT", [128, d], bf16, kind="ExternalInput").ap()
    ident_in = nc.dram_tensor("ident", [128, 128], f32,
                              kind="ExternalInput").ap()
    out = nc.dram_tensor("out", [d, q], bf16, kind="ExternalOutput").ap()

    with tile.TileContext(nc) as tc, ExitStack() as ctx:
        # ---------------- persistent tiles ----------------
        kt_pool = ctx.enter_context(tc.tile_pool(name="kt", bufs=1))
        kn_pool = ctx.enter_context(tc.tile_pool(name="kn", bufs=1))
        qm_pool = ctx.enter_context(tc.tile_pool(name="qm", bufs=nqt))
        qt_pool = ctx.enter_context(tc.tile_pool(name="qtb", bufs=nqb))
        w_pool = ctx.enter_context(tc.tile_pool(name="w", bufs=1))

        # K^T per head, other head's rows zeroed (full-128 contraction:
        # sub-tile base-partition matmuls crash the PE exec unit on HW)
        ktp = [kt_pool.tile([128, k], bf16, tag=f"ktp{h}", name=f"ktp{h}")
               for h in range(2)]
        nc.vector.memset(ktp[0][64:128, :], 0.0)
        nc.vector.memset(ktp[1][0:64, :], 0.0)
        # K natural + tens col, DoubleRow layout [k%128, pair, r, hd+1]
        knat = [kn_pool.tile([128, npair, 2, 65], f8e4, tag=f"knat{h}",
                             name=f"knat{h}")
                for h in range(2)]
        for h in range(2):
            nc.vector.memset(knat[h][:], 10.0)
        # master q tiles [q-tile 128, hd-pair 128] fp32
        qM = [qm_pool.tile([128, 128], f32, tag="qm", name=f"qm{i}")
              for i in range(nqt)]
        # scores rhs qT bf16, one tile per q-block for slice-level dep tracking
        qTb = [qt_pool.tile([128, QB], bf16, tag="qtb", name=f"qtb{j}")
               for j in range(nqb)]

        wk_sb = w_pool.tile([128, d], bf16, tag="wk")
        wq_sb = w_pool.tile([128, d], bf16, tag="wq")
        wo_sb = w_pool.tile([128, d], bf16, tag="wo")
        ident = w_pool.tile([128, 128], f32, tag="id")

        for c in range(ndc):
            cs = slice(c * 128, (c + 1) * 128)
            nc.sync.dma_start(out=wk_sb[:, cs], in_=wk[cs, :])
            nc.scalar.dma_start(out=wq_sb[:, cs], in_=wq[cs, :])
        nc.scalar.dma_start(out=wo_sb[:], in_=woT[:])
        nc.scalar.dma_start(out=ident[:], in_=ident_in[:])

        # ---------------- phase A: projections ----------------
        with tc.tile_pool(name="ctxp", bufs=ndc) as ctx_pool, \
             tc.tile_pool(name="tgtp", bufs=ndc) as tgt_pool, \
             tc.tile_pool(name="psA", bufs=2, space="PSUM") as psA, \
             tc.tile_pool(name="psB", bufs=3, space="PSUM") as psB, \
             tc.tile_pool(name="psQ", bufs=2, space="PSUM") as psQ, \
             tc.tile_pool(name="psT", bufs=1, space="PSUM") as psT:
            ctx_tiles = [ctx_pool.tile([128, k], f8e4, tag="ctx",
                                       name=f"ctx{c}") for c in range(ndc)]
            tgt_tiles = [tgt_pool.tile([128, q], bf16, tag="tgt",
                                       name=f"tgt{c}") for c in range(ndc)]
            for c in range(ndc):
                cs = slice(c * 128, (c + 1) * 128)
                nc.sync.dma_start(out=ctx_tiles[c][:], in_=ctxT[cs, :])
            for c in range(ndc):
                cs = slice(c * 128, (c + 1) * 128)
                nc.sync.dma_start(out=tgt_tiles[c][:], in_=tgtT[cs, :])

            # K^T = Wk_pair^T @ context^T  -> ktp (bf16)
            for kb in range(nkb):
                ks = slice(kb * 512, (kb + 1) * 512)
                pk = psA.tile([128, 512], f32, tag="pk")
                for c in range(ndc):
                    cs = slice(c * 128, (c + 1) * 128)
                    nc.tensor.matmul(out=pk[:], lhsT=wk_sb[:, cs],
                                     rhs=ctx_tiles[c][:, ks],
                                     start=(c == 0), stop=(c == ndc - 1))
                nc.vector.tensor_copy(out=ktp[0][0:64, ks], in_=pk[0:64, :])
                nc.scalar.copy(out=ktp[1][64:128, ks], in_=pk[64:128, :])

            # K natural (fp8, DoubleRow layout), pair-granular psum;
            # col 64 stays 10.0
            for pair in range(npair):
                pn = psB.tile([128, 2, 128], f32, tag="pn")
                for r in range(2):
                    ks = slice((2 * pair + r) * 128, (2 * pair + r + 1) * 128)
                    for c in range(ndc):
                        cs = slice(c * 128, (c + 1) * 128)
                        nc.tensor.matmul(out=pn[:, r, :],
                                         lhsT=ctx_tiles[c][:, ks],
                                         rhs=wk_sb[:, cs],
                                         start=(c == 0), stop=(c == ndc - 1))
                nc.vector.tensor_copy(out=knat[0][:, pair, :, 0:64],
                                      in_=pn[:, :, 0:64])
                nc.scalar.copy(out=knat[1][:, pair, :, 0:64],
                               in_=pn[:, :, 64:128])

            # Q projection, transposed orientation: out [q-tile, hd] fp32
            for i in range(nqt):
                qs = slice(i * 128, (i + 1) * 128)
                pq = psQ.tile([128, 128], f32, tag="pq")
                for c in range(ndc):
                    cs = slice(c * 128, (c + 1) * 128)
                    nc.tensor.matmul(out=pq[:], lhsT=tgt_tiles[c][:, qs],
                                     rhs=wq_sb[:, cs],
                                     start=(c == 0), stop=(c == ndc - 1))
                if i % 2 == 0:
                    nc.vector.tensor_copy(out=qM[i][:], in_=pq[:])
                else:
                    nc.scalar.copy(out=qM[i][:], in_=pq[:])
                # qT init via PE transpose
                pt = psT.tile([128, 128], f32, tag="pt")
                nc.tensor.transpose(pt[:], qM[i][:], ident[:])
                if i % 2 == 0:
                    nc.scalar.copy(out=qTb[i // 2][:, (i % 2) * 128:
                                                   (i % 2 + 1) * 128],
                                   in_=pt[:])
                else:
                    nc.vector.tensor_copy(out=qTb[i // 2][:, (i % 2) * 128:
                                                          (i % 2 + 1) * 128],
                                          in_=pt[:])

        # ---------------- phase B: energy steps ----------------
        # Per-engine private PSUM rings: ACT rings 2x[128,2,512] (one tile
        # per k-pair), DVE rings 2x[128,512] (one per k-chunk), so each exp
        # engine runs back-to-back without coupling through a shared ring.
        # gt packs 4 grad accumulators (2h x 2qt x 65) in one bank at
        # col (2h+qt)*65; cols 260:388 are PE-transpose scratch.
        with tc.tile_pool(name="pp", bufs=5) as p_pool, \
             tc.tile_pool(name="rr", bufs=4) as r_pool, \
             tc.tile_pool(name="ps_sa", bufs=2, space="PSUM") as ps_sa, \
             tc.tile_pool(name="ps_sd", bufs=2, space="PSUM") as ps_sd, \
             tc.tile_pool(name="ps_g", bufs=2, space="PSUM") as ps_g:

            def make_update_ops(j, gt):
                """Update ops for q-block j (reads gt), emitted interleaved
                into the next q-block's chunk loop."""
                ops = []
                trcs = []
                for qt in range(QB // 128):
                    i = j * (QB // 128) + qt
                    for h in range(2):
                        hs = slice(h * 64, (h + 1) * 64)
                        off = (h * (QB // 128) + qt) * 65

                        def upd(i=i, hs=hs, off=off):
                            rt = r_pool.tile([128, 1], f32, tag="r",
                                             name="rt")
                            nc.vector.reciprocal(
                                out=rt[:], in_=gt[:, off + 64:off + 65])
                            nc.vector.scalar_tensor_tensor(
                                out=qM[i][:, hs], in0=gt[:, off:off + 64],
                                scalar=rt[:], in1=qM[i][:, hs],
                                op0=MULT, op1=ADD)
                        ops.append(upd)

                    def trc(j=j, qt=qt, i=i):
                        pt = gt[:, 260:388]
                        nc.tensor.transpose(pt, qM[i][:], ident[:])
                        if qt % 2 == 0:
                            nc.scalar.copy(
                                out=qTb[j][:, qt * 128:(qt + 1) * 128],
                                in_=pt)
                        else:
                            nc.vector.tensor_copy(
                                out=qTb[j][:, qt * 128:(qt + 1) * 128],
                                in_=pt)
                    trcs.append(trc)
                if "notrc" in dbg:
                    return ops
                return ops + trcs

            def emit_grads(gt, p_t, pair):
                # One PSUM accumulation group per bank: only the first
                # accumulator starts it (start zeroes the whole 2KB zero
                # region; each other accumulator's first write lands on
                # pending-zero bytes and overwrites), only the last stops.
                nq = QB // 128
                for h in range(2):
                    for qt in range(nq):
                        qs = slice(h * QB + qt * 128,
                                   h * QB + (qt + 1) * 128)
                        off = (h * nq + qt) * 65
                        first = h == 0 and qt == 0
                        last = h == 1 and qt == nq - 1
                        nc.tensor.matmul(
                            out=gt[:, off:off + 65],
                            lhsT=p_t[:, :, qs],
                            rhs=knat[h][:, pair],
                            start=(pair == 0 and first),
                            stop=(pair == npair - 1 and last),
                            perf_mode=DR)

            pending = []
            for t in range(steps):
                for j in range(nqb):
                    gt = ps_g.tile([128, 512], f32, tag="g",
                                   name=f"g{t}_{j}")
                    grad_backlog = []
                    for pair in range(npair):
                        p_t = p_pool.tile([128, 2, 2 * QB], f8e4, tag="p",
                                          name="p_t")
                        if pair in dve_set:
                            for r in range(2):
                                kc = 2 * pair + r
                                ks = slice(kc * 128, (kc + 1) * 128)
                                sd = ps_sd.tile([128, 2 * QB], f32, tag="sd")
                                for h in range(2):
                                    nc.tensor.matmul(
                                        out=sd[:, h * QB:(h + 1) * QB],
                                        lhsT=ktp[h][:, ks],
                                        rhs=qTb[j][:],
                                        start=True, stop=True)
                                nc.vector.tensor_scalar(
                                    out=p_t[:, r, :].bitcast(i8), in0=sd[:],
                                    scalar1=A8, scalar2=B8,
                                    op0=MULT, op1=ADD)
                        else:
                            sa = ps_sa.tile([128, 2, 2 * QB], f32, tag="sa")
                            for r in range(2):
                                kc = 2 * pair + r
                                ks = slice(kc * 128, (kc + 1) * 128)
                                for h in range(2):
                                    nc.tensor.matmul(
                                        out=sa[:, r, h * QB:(h + 1) * QB],
                                        lhsT=ktp[h][:, ks],
                                        rhs=qTb[j][:],
                                        start=True, stop=True)
                            nc.scalar.activation(p_t[:], sa[:], EXP,
                                                 scale=beta)
                        # delayed grads so PE never waits on a fresh exp
                        if "nograd" not in dbg:
                            grad_backlog.append((p_t, pair))
                            if len(grad_backlog) > grad_delay:
                                emit_grads(gt, *grad_backlog.pop(0))
                        for _ in range(pop_rate):
                            if pending:
                                pending.pop(0)()
                    for bl in grad_backlog:
                        emit_grads(gt, *bl)
                    while pending:
                        pending.pop(0)()
                    if "noupd" not in dbg and "nograd" not in dbg:
                        pending = make_update_ops(j, gt)
            while pending:
                pending.pop(0)()

        # ---------------- phase C: output projection ----------------
        # out^T [d, q] = Wo_block^T... out[dt, qs] = woT[:, dt]^T @ qT
        with tc.tile_pool(name="fo", bufs=6) as fo_pool, \
             tc.tile_pool(name="psO", bufs=4, space="PSUM") as psO:
            for dt in range(d // 128):
                ds_ = slice(dt * 128, (dt + 1) * 128)
                for jb in range(nqb // 2):
                    po = psO.tile([128, 512], f32, tag="po")
                    for u in range(2):
                        nc.tensor.matmul(out=po[:, u * QB:(u + 1) * QB],
                                         lhsT=wo_sb[:, ds_],
                                         rhs=qTb[2 * jb + u][:],
                                         start=True, stop=True)
                    ot = fo_pool.tile([128, 512], bf16, tag="ot")
                    if (dt * (nqb // 2) + jb) % 2 == 0:
                        nc.vector.tensor_copy(out=ot[:], in_=po[:])
                    else:
                        nc.scalar.copy(out=ot[:], in_=po[:])
                    nc.sync.dma_start(
                        out=out[ds_, jb * 512:(jb + 1) * 512], in_=ot[:])

    nc.compile()
    return nc


def _get_program():
    if "nc" not in _CACHE:
        _CACHE["nc"] = build_program()
    return _CACHE["nc"]


def make_in_maps(context, target_init, Wq, Wk, Wo):
    """Host-side sharding/layout prep: one input map per core."""
    ctxT = np.ascontiguousarray(context.T).astype(F8E4)        # [D, K]
    tgtT = np.ascontiguousarray(target_init.T).astype(BF16)    # [D, Q]
    ident = np.eye(128, dtype=np.float32)
    in_maps = []
    for c in range(N_CORES):
        h0, h1 = 2 * c, 2 * c + 1
        wk_c = np.concatenate([Wk[h0].T, Wk[h1].T], axis=1)    # [D, 128]
        wq_c = np.concatenate([Wq[h0].T, Wq[h1].T], axis=1)    # [D, 128]
        woT_c = np.ascontiguousarray(Wo[:, 128 * c:128 * (c + 1)].T)
        in_maps.append({
            "ctxT": ctxT,
            "tgtT": tgtT,
            "wk": np.ascontiguousarray(wk_c).astype(BF16),
            "wq": np.ascontiguousarray(wq_c).astype(BF16),
            "woT": woT_c.astype(BF16),
            "ident": ident,
        })
    return in_maps


def kernel(context, target_init, Wq, Wk, Wo):
    context = np.asarray(context, dtype=np.float32)
    target_init = np.asarray(target_init, dtype=np.float32)
    Wq = np.asarray(Wq, dtype=np.float32)
    Wk = np.asarray(Wk, dtype=np.float32)
    Wo = np.asarray(Wo, dtype=np.float32)

    in_maps = make_in_maps(context, target_init, Wq, Wk, Wo)

    last_err = None
    for _attempt in range(3):
        try:
            results = _run_spmd(in_maps)
            break
        except Exception as e:  # transient axon RESOURCE_EXHAUSTED etc.
            last_err = e
            _CACHE.clear()
    else:
        raise last_err

    acc = np.zeros((D, Q), dtype=np.float32)
    for c in range(N_CORES):
        acc += results[c]["out"].astype(np.float32)
    return np.ascontiguousarray(acc.T)


def _run_spmd(in_maps):
    """Run the program on cores 0..7 with a cached jitted executable."""
    nc = _get_program()
    try:
        runner = _CACHE.get("runner")
        if runner is None:
            runner = _SpmdRunner(nc, N_CORES)
            _CACHE["runner"] = runner
        return runner.run(in_maps)
    except Exception:
        _CACHE.pop("runner", None)
        from concourse.bass_utils import run_bass_kernel_spmd
        res = run_bass_kernel_spmd(nc, in_maps, list(range(N_CORES)))
        return res.results


class _SpmdRunner:
    """Persistent jitted shard_map executable (no output donation so the
    executable and zero buffers are reusable across calls)."""

    def __init__(self, nc, n_cores):
        import jax
        from jax.experimental.shard_map import shard_map
        from jax.sharding import Mesh, NamedSharding, PartitionSpec
        import concourse.mybir as mybir
        from concourse.bass2jax import (
            _bass_exec_p, install_neuronx_cc_hook, partition_id_tensor)

        install_neuronx_cc_hook()
        self.jax = jax
        self.n_cores = n_cores
        partition_name = (nc.partition_id_tensor.name
                          if nc.partition_id_tensor else None)
        in_names, out_names, out_avals, zero_outs = [], [], [], []
        for alloc in nc.m.functions[0].allocations:
            if not isinstance(alloc, mybir.MemoryLocationSet):
                continue
            name = alloc.memorylocations[0].name
            if alloc.kind == "ExternalInput":
                if name != partition_name:
                    in_names.append(name)
            elif alloc.kind == "ExternalOutput":
                shape = tuple(alloc.tensor_shape)
                dtype = mybir.dt.np(alloc.dtype)
                out_names.append(name)
                out_avals.append(jax.core.ShapedArray(shape, dtype))
                zero_outs.append(np.zeros(shape, dtype))
        self.in_names = in_names
        self.out_names = out_names
        self.out_avals = out_avals
        all_in_names = in_names + out_names
        if partition_name is not None:
            all_in_names.append(partition_name)

        def _body(*args):
            operands = list(args)
            if partition_name is not None:
                operands.append(partition_id_tensor())
            outs = _bass_exec_p.bind(
                *operands,
                out_avals=tuple(out_avals),
                in_names=tuple(all_in_names),
                out_names=tuple(out_names),
                lowering_input_output_aliases=(),
                sim_require_finite=True,
                sim_require_nnan=True,
                nc=nc,
            )
            return tuple(outs)

        devices = jax.devices()[:n_cores]
        mesh = Mesh(np.asarray(devices), ("core",))
        in_specs = (PartitionSpec("core"),) * (len(in_names) + len(out_names))
        out_specs = (PartitionSpec("core"),) * len(out_names)
        self.fn = jax.jit(
            shard_map(_body, mesh=mesh, in_specs=in_specs,
                      out_specs=out_specs, check_rep=False),
            keep_unused=True,
        )
        self.sharding = NamedSharding(mesh, PartitionSpec("core"))
        self.zeros_placed = [
            jax.device_put(np.concatenate([z] * n_cores, axis=0),
                           self.sharding)
            for z in zero_outs
        ]

    def place(self, in_maps):
        concat = [
            np.concatenate([np.asarray(in_maps[c][n])
                            for c in range(self.n_cores)], axis=0)
            for n in self.in_names
        ]
        return [self.jax.device_put(a, self.sharding) for a in concat]

    def exec_placed(self, placed):
        outs = self.fn(*placed, *self.zeros_placed)
        self.jax.block_until_ready(outs)
        return outs

    def run(self, in_maps):
        outs = self.exec_placed(self.place(in_maps))
        per_core = []
        for c in range(self.n_cores):
            d = {}
            for i, n in enumerate(self.out_names):
                full = np.asarray(outs[i])
                sh = self.out_avals[i].shape
                d[n] = full.reshape(self.n_cores, *sh)[c]
            per_core.append(d)
        return per_core
